# revision 6
# baseline (speedup 1.0000x reference)
"""Batched per-sample MLP heads (MoE-routing style) on 8 TRN2 NeuronCores.

y[b] = W2[a] @ relu(W1[a] @ h[b] + b1[a]) + b2[a],  a = asset_ids[b]

Strategy (expert-parallel):
  * Host groups samples by asset into "slots" of <=32 samples, distributes
    slots round-robin over the 8 cores, and pre-packs all tensors into
    DMA-friendly per-core layouts (W1 transposed so the contraction dim d
    lands on SBUF partitions).
  * Device streams each slot's 1MB W1^T tile once and runs small fp32
    matmuls with the slot's h^T columns as the stationary operand
    (samples on PSUM partitions, hidden on the free dim).  Four slots are
    packed into one [128, 1024] PSUM block via tile_position col-tiling.
    b1 and W2 rows are broadcast into PSUM with a K=4 matmul against a
    0/1 selection matrix.  Epilogue: ACT relu (PSUM->SBUF), DVE mul by
    W2, DVE reduce along hidden, b2 add, DMA out.
  * The program is compiled per asset_ids distribution (slot counts are
    baked in); the harness's inputs are deterministic so in practice this
    compiles once.
"""

import numpy as np

N_ASSETS, D_MODEL, HIDDEN, BATCH = 1024, 256, 1024, 2048
N_CORES = 8
C = 32        # sample lanes per slot
SPB = 4       # slots per PSUM block (4 * 32 = 128 partitions)
DH = D_MODEL // 128          # d-halves (2)
SLOT_F = DH * HIDDEN         # free-dim floats per slot in the W1 tile (2048)

_prog_cache: dict = {}


def _plan(asset_ids: np.ndarray):
    """Group samples by asset into slots of <= C samples, assign to cores."""
    asset_ids = np.asarray(asset_ids).astype(np.int64).ravel()
    B = asset_ids.shape[0]
    order = np.argsort(asset_ids, kind="stable")
    aid_sorted = asset_ids[order]
    slots = []  # (asset, orig sample indices)
    start = 0
    while start < B:
        a = aid_sorted[start]
        end = start
        while end < B and aid_sorted[end] == a:
            end += 1
        for s in range(start, end, C):
            slots.append((int(a), order[s:min(s + C, end)]))
        start = end
    per_core = [slots[c::N_CORES] for c in range(N_CORES)]
    s_max = max(len(p) for p in per_core)
    nblk = (s_max + SPB - 1) // SPB
    return per_core, nblk


def _pack(h, W1, b1, W2, b2, core_slots, nblk):
    """Build the per-core input arrays for one core."""
    S = nblk * SPB
    n_real = len(core_slots)
    assets = np.zeros(S, dtype=np.int64)
    for j, (a, _) in enumerate(core_slots):
        assets[j] = a

    # W1 tile: arr[b, p, jj*2048 + dh*1024 + hh] = W1[a_slot, hh, dh*128+p]
    g = W1[assets[:n_real]]                       # [n_real, 1024, 256]
    gg = np.zeros((S, DH, 128, HIDDEN), dtype=np.float32)
    gg[:n_real] = g.transpose(0, 2, 1).reshape(n_real, DH, 128, HIDDEN)
    w1t = np.ascontiguousarray(
        gg.reshape(nblk, SPB, DH, 128, HIDDEN)
        .transpose(0, 3, 1, 2, 4)
        .reshape(nblk, 128, SPB * SLOT_F)
    )

    # h^T columns: htg[p, dh, lane] = h[sample(lane), dh*128+p]
    hcols = np.zeros((S * C, D_MODEL), dtype=np.float32)
    for j, (_, samp) in enumerate(core_slots):
        hcols[j * C:j * C + len(samp)] = h[samp]
    htg = np.ascontiguousarray(
        hcols.T.reshape(DH, 128, S * C).transpose(1, 0, 2)
    )

    b1r = np.zeros((S, HIDDEN), dtype=np.float32)
    b1r[:n_real] = b1[assets[:n_real]]
    w2r = np.zeros((S, HIDDEN), dtype=np.float32)
    w2r[:n_real] = W2[assets[:n_real], 0, :]
    b2s = np.zeros(S, dtype=np.float32)
    b2s[:n_real] = b2[assets[:n_real], 0]
    b2g = np.ascontiguousarray(np.repeat(b2s, C).reshape(nblk, 128).T)

    ee = np.zeros((SPB, 128), dtype=np.float32)
    for j in range(SPB):
        ee[j, j * C:(j + 1) * C] = 1.0

    return {"w1t": w1t, "htg": htg, "b1r": b1r, "w2r": w2r,
            "b2g": b2g, "ee": ee}


def _build(nblk: int, repeat: int = 1):
    """Build + compile the SPMD program for a given block count."""
    import concourse.tile as tile
    from concourse import bacc, mybir

    key = (nblk, repeat)
    if key in _prog_cache:
        return _prog_cache[key]

    S = nblk * SPB
    f32 = mybir.dt.float32
    nc = bacc.Bacc(None, target_bir_lowering=False, debug=False)
    w1t = nc.dram_tensor("w1t", [nblk, 128, SPB * SLOT_F], f32, kind="ExternalInput")
    htg = nc.dram_tensor("htg", [128, DH, S * C], f32, kind="ExternalInput")
    b1r = nc.dram_tensor("b1r", [S, HIDDEN], f32, kind="ExternalInput")
    w2r = nc.dram_tensor("w2r", [S, HIDDEN], f32, kind="ExternalInput")
    b2g = nc.dram_tensor("b2g", [128, nblk], f32, kind="ExternalInput")
    ee = nc.dram_tensor("ee", [SPB, 128], f32, kind="ExternalInput")
    out = nc.dram_tensor("out", [128, nblk], f32, kind="ExternalOutput")

    with tile.TileContext(nc) as tc:
        with (
            tc.tile_pool(name="singles", bufs=1) as singles,
            tc.tile_pool(name="w1pool", bufs=3) as w1pool,
            tc.tile_pool(name="small", bufs=3) as small,
            tc.tile_pool(name="zpsum", bufs=2, space="PSUM") as zpsum,
            tc.tile_pool(name="wpsum", bufs=2, space="PSUM") as wpsum,
            tc.tile_pool(name="sb", bufs=2) as sb,
        ):
            htg_t = singles.tile([128, DH, S * C], f32)
            nc.sync.dma_start(out=htg_t[:], in_=htg[:])
            ee_t = singles.tile([SPB, 128], f32)
            nc.sync.dma_start(out=ee_t[:], in_=ee[:])
            b2_t = singles.tile([128, nblk], f32)
            nc.sync.dma_start(out=b2_t[:], in_=b2g[:])
            z2_t = singles.tile([128, nblk], f32)

            for _rep in range(repeat):
                for b in range(nblk):
                    w1_t = w1pool.tile([128, SPB * SLOT_F], f32, tag="w1")
                    nc.sync.dma_start(out=w1_t[:], in_=w1t[b])
                    b1_t = small.tile([SPB, HIDDEN], f32, tag="b1")
                    nc.sync.dma_start(out=b1_t[:], in_=b1r[b * SPB:(b + 1) * SPB, :])
                    w2_t = small.tile([SPB, HIDDEN], f32, tag="w2")
                    nc.sync.dma_start(out=w2_t[:], in_=w2r[b * SPB:(b + 1) * SPB, :])

                    zps = [zpsum.tile([128, 512], f32, tag=f"z{bank}", name=f"zps{bank}")
                           for bank in range(2)]
                    wps = [wpsum.tile([128, 512], f32, tag=f"w{bank}", name=f"wps{bank}")
                           for bank in range(2)]
                    for bank in range(2):
                        bs = slice(bank * 512, (bank + 1) * 512)
                        # broadcast b1 rows of the 4 slots into the bank
                        nc.tensor.matmul(
                            zps[bank][:], lhsT=ee_t[:], rhs=b1_t[:, bs],
                            start=True, stop=True,
                        )
                        # broadcast w2 rows (own PSUM block)
                        nc.tensor.matmul(
                            wps[bank][:], lhsT=ee_t[:], rhs=w2_t[:, bs],
                            start=True, stop=True,
                        )
                        for jj in range(SPB):
                            for dh in range(DH):
                                lane0 = (b * SPB + jj) * C
                                last = dh == DH - 1
                                nc.tensor.matmul(
                                    zps[bank][32 * jj:32 * (jj + 1), :],
                                    lhsT=htg_t[:, dh, lane0:lane0 + C],
                                    rhs=w1_t[:, jj * SLOT_F + dh * HIDDEN + bank * 512:
                                             jj * SLOT_F + dh * HIDDEN + (bank + 1) * 512],
                                    start=False, stop=last,
                                    tile_position=(0, 32 * jj),
                                    skip_group_check=True,
                                )
                    a1 = sb.tile([128, HIDDEN], f32, tag="a1")
                    t2 = sb.tile([128, HIDDEN], f32, tag="t2")
                    for bank in range(2):
                        bs = slice(bank * 512, (bank + 1) * 512)
                        nc.scalar.activation(
                            out=a1[:, bs], in_=zps[bank][:],
                            func=mybir.ActivationFunctionType.Relu,
                        )
                        nc.vector.tensor_mul(t2[:, bs], a1[:, bs], wps[bank][:])
                    nc.vector.tensor_reduce(
                        out=z2_t[:, b:b + 1], in_=t2[:],
                        axis=mybir.AxisListType.X, op=mybir.AluOpType.add,
                    )
            nc.vector.tensor_add(z2_t[:], z2_t[:], b2_t[:])
            nc.sync.dma_start(out=out[:], in_=z2_t[:])
    nc.compile()
    _prog_cache[key] = nc
    return nc


def _run(in_maps, nc):
    from concourse.bass_utils import run_bass_kernel_spmd
    res = run_bass_kernel_spmd(nc, in_maps, core_ids=list(range(N_CORES)))
    return res.results


def prepare(h, asset_ids, W1, b1, W2, b2, repeat: int = 1):
    """Host-side planning/packing + program build. Returns (nc, in_maps, plan)."""
    h = np.asarray(h, dtype=np.float32)
    W1 = np.asarray(W1, dtype=np.float32)
    b1 = np.asarray(b1, dtype=np.float32)
    W2 = np.asarray(W2, dtype=np.float32)
    b2 = np.asarray(b2, dtype=np.float32)
    per_core, nblk = _plan(asset_ids)
    in_maps = [_pack(h, W1, b1, W2, b2, per_core[c], nblk) for c in range(N_CORES)]
    nc = _build(nblk, repeat=repeat)
    return nc, in_maps, per_core


def unpack_outputs(results, per_core, batch):
    y = np.zeros(batch, dtype=np.float32)
    for c in range(N_CORES):
        o = results[c]["out"]  # [128, nblk]
        for j, (_, samp) in enumerate(per_core[c]):
            b, jj = divmod(j, SPB)
            y[samp] = o[jj * C:jj * C + len(samp), b]
    return y


def kernel(h, asset_ids, W1, b1, W2, b2):
    nc, in_maps, per_core = prepare(h, asset_ids, W1, b1, W2, b2, repeat=1)
    results = _run(in_maps, nc)
    return unpack_outputs(results, per_core, np.asarray(h).shape[0])
